# revision 1
# baseline (speedup 1.0000x reference)
"""Distributed multi-head attention kernel for 8 TRN2 NeuronCores.

Sharding: tensor-parallel over heads (2 heads/core) for the qkv projection
and attention; AllToAll exchange of the attention output; row-parallel output
projection (each core produces a transposed 512-row slice of the final
output); host reassembles.

Compute in bf16 on the PE array (f32 PSUM accumulation everywhere, f32
softmax denominators/normalization). The host pre-transposes x to [dim, b*s]
and pre-casts x/wqkv/wo to bf16 as part of sharding/layout prep.
"""

import sys

sys.path.insert(0, "/opt/trn_rl_repo")

import ml_dtypes
import numpy as np

# Problem constants (hardcoded per harness contract)
B = 2
S = 2048
DIM = 1024
N_HEAD = 16
HD = 64  # head dim
SCALE = HD ** (-0.5)
R = B * S  # 4096 flattened rows
NCORES = 8
HPC = N_HEAD // NCORES  # 2 heads per core
FPC = HPC * HD  # 128 features per core
RPC = R // NCORES  # 512 rows per core (output row slice)

KT = DIM // 128  # 8 k-tiles over the model dim
NKT = S // 128  # 16 key tiles per sequence
NQB = S // 512  # 4 query blocks per sequence

_CACHED = {}


def _build_graph():
    import concourse.mybir as mybir
    import concourse.tile as tile
    from concourse import bacc
    from concourse.masks import make_identity

    # This kernel only uses Exp and Ln activations. Both live in the
    # "natural_log_exp_and_others" ACT table set, but the table-load pass
    # maps each function to the first set containing it, which puts Exp in
    # "exp_and_others" and forces a ~1.3us table reload around every Ln.
    # Restrict the table list for this build so both resolve to the same
    # set and the table loads exactly once.
    real_get_tables = bacc.get_activation_tables

    def _tables_ln_exp_merged(arch):
        # Keep dict size/order (act_func_set_id is positional into
        # act_info.json); just make "exp_and_others" unselectable.
        tables = dict(real_get_tables(arch))
        if "natural_log_exp_and_others" in tables and "exp_and_others" in tables:
            tables["exp_and_others"] = set()
        return tables

    nc = bacc.Bacc(
        "TRN2",
        target_bir_lowering=False,
        debug=False,
        num_devices=NCORES,
    )

    bacc.get_activation_tables = _tables_ln_exp_merged
    try:
        _nc = _build_body(nc, mybir, tile, make_identity)
    finally:
        bacc.get_activation_tables = real_get_tables
    return _nc


def _build_body(nc, mybir, tile, make_identity):
    f32 = mybir.dt.float32
    f32r = mybir.dt.float32r
    bf16 = mybir.dt.bfloat16
    EXP = mybir.ActivationFunctionType.Exp
    LN = mybir.ActivationFunctionType.Ln

    xt = nc.dram_tensor("xt", [DIM, R], bf16, kind="ExternalInput").ap()
    wqkv = nc.dram_tensor("wqkv", [DIM, 3 * FPC], bf16, kind="ExternalInput").ap()
    bqkv = nc.dram_tensor("bqkv", [3, FPC], f32, kind="ExternalInput").ap()
    wo = nc.dram_tensor("wo", [DIM, DIM], bf16, kind="ExternalInput").ap()
    bo = nc.dram_tensor("bo", [8, 128], f32, kind="ExternalInput").ap()
    out = nc.dram_tensor("out", [DIM, RPC], f32, kind="ExternalOutput").ap()

    with tile.TileContext(nc) as tc:
        with tc.tile_pool(name="glob", bufs=1) as glob:
            ident16 = glob.tile([128, 128], bf16)
            make_identity(nc, ident16)
            # biases as [128, 1] per-partition vectors (f32); emitted after
            # the weight loads (slow 128-descriptor SWDGE transfers)
            bias_qkv = glob.tile([128, 3], f32)
            bias_o = glob.tile([128, 8], f32)

            # persistent activations (bf16 compute operands)
            qT = glob.tile([128, R], bf16)  # [2 heads x 64 d, b*s]
            kT = glob.tile([128, R], bf16)
            # v natural layout + ones column: per (b, h, kt) a [128, 65] block
            v_nat = glob.tile([128, B * HPC * NKT * 65], bf16)
            ones_tmp = glob.tile([128, 64], f32)
            nc.vector.memset(ones_tmp[:], 1.0)
            nc.vector.tensor_copy(out=v_nat[:, 64::65], in_=ones_tmp[:])
            # attention output, transposed, bf16, one tile per head
            oT = [glob.tile([64, R], bf16, name=f"oT_{hh}") for hh in range(HPC)]

            # ---------------- stage A: qkv projection ---------------------
            with (
                tc.tile_pool(name="xTp", bufs=2) as xT_pool,
                tc.tile_pool(name="wq", bufs=1) as wq_pool,
                tc.tile_pool(name="vt", bufs=2) as vt_pool,
                tc.tile_pool(name="ps_tr", bufs=2, space="PSUM") as ps_tr,
                tc.tile_pool(name="ps_proj", bufs=1, space="PSUM") as ps_proj,
            ):
                wqkv_sb = []
                for k in range(KT):
                    w_t = wq_pool.tile([128, 3 * FPC], bf16, name=f"w_{k}")
                    nc.gpsimd.dma_start(
                        out=w_t[:], in_=wqkv[k * 128 : (k + 1) * 128, :]
                    )
                    wqkv_sb.append(w_t)
                for m in range(3):
                    nc.gpsimd.dma_start(
                        out=bias_qkv[:, m : m + 1], in_=bqkv[m : m + 1, :]
                    )
                for m in range(8):
                    nc.gpsimd.dma_start(out=bias_o[:, m : m + 1], in_=bo[m : m + 1, :])

                NG = 4  # row groups of 1024
                for g in range(NG):
                    xT_g = [
                        xT_pool.tile([128, 1024], bf16, name=f"xT_{k}", tag=f"xT{k}")
                        for k in range(KT)
                    ]
                    for k in range(KT):
                        nc.sync.dma_start(
                            out=xT_g[k][:],
                            in_=xt[k * 128 : (k + 1) * 128, g * 1024 : (g + 1) * 1024],
                        )
                    # projection for this row group; 6 psum banks (q/k/v x 2)
                    pp = [
                        [
                            ps_proj.tile(
                                [128, 512], f32, name=f"pp_{m}_{h}", tag=f"pp{m}{h}"
                            )
                            for h in range(2)
                        ]
                        for m in range(3)
                    ]
                    for k in range(KT):
                        for m in range(3):
                            for h in range(2):
                                nc.tensor.matmul(
                                    pp[m][h][:],
                                    lhsT=wqkv_sb[k][:, m * 128 : (m + 1) * 128],
                                    rhs=xT_g[k][:, h * 512 : (h + 1) * 512],
                                    start=(k == 0),
                                    stop=(k == KT - 1),
                                )
                    for h in range(2):
                        col0 = g * 1024 + h * 512
                        nc.vector.tensor_scalar_add(
                            out=qT[:, col0 : col0 + 512],
                            in0=pp[0][h][:],
                            scalar1=bias_qkv[:, 0:1],
                        )
                        nc.vector.tensor_scalar_add(
                            out=kT[:, col0 : col0 + 512],
                            in0=pp[1][h][:],
                            scalar1=bias_qkv[:, 1:2],
                        )
                        # v: bias (-> bf16), then transpose to natural layout
                        vT_blk = vt_pool.tile(
                            [128, 512], bf16, name="vT_blk", tag="vT_blk"
                        )
                        nc.vector.tensor_scalar_add(
                            out=vT_blk[:], in0=pp[2][h][:], scalar1=bias_qkv[:, 2:3]
                        )
                        for j in range(4):
                            row0 = col0 + j * 128
                            b_idx = row0 // S
                            kt = (row0 % S) // 128
                            for hh in range(HPC):
                                pstv = ps_tr.tile(
                                    [128, 64], bf16, name="pstv", tag="pstv"
                                )
                                nc.tensor.transpose(
                                    pstv[:],
                                    vT_blk[
                                        hh * 64 : (hh + 1) * 64,
                                        j * 128 : (j + 1) * 128,
                                    ],
                                    ident16[
                                        hh * 64 : (hh + 1) * 64,
                                        hh * 64 : (hh + 1) * 64,
                                    ],
                                )
                                col = ((b_idx * HPC + hh) * NKT + kt) * 65
                                nc.vector.tensor_copy(
                                    out=v_nat[:, col : col + 64], in_=pstv[:]
                                )

            # ---------------- stage C: attention --------------------------
            with (
                tc.tile_pool(name="dram", bufs=1, space="DRAM") as dram_pool,
                tc.tile_pool(name="ps_st", bufs=2, space="PSUM") as ps_st,
                tc.tile_pool(name="ps_o", bufs=2, space="PSUM") as ps_o,
                tc.tile_pool(name="ptp", bufs=6) as pt_pool,
                tc.tile_pool(name="nrm", bufs=3) as nrm_pool,
                tc.tile_pool(name="onesp", bufs=1) as ones_pool,
            ):
                # ones row living on partition 64 (lane-aligned with po row 64)
                ones64 = ones_pool.tile([65, 64], f32r)
                nc.vector.tensor_copy(out=ones64[64:65, :], in_=ones_tmp[64:65, :])
                warm_in = dram_pool.tile([NCORES, 16], bf16, name="warm_in")
                warm_out = dram_pool.tile([NCORES, 16], bf16, name="warm_out")
                nc.gpsimd.dma_start(out=warm_in[0:1, :], in_=ones_tmp[0:1, 0:16])
                nc.gpsimd.dma_start(
                    out=warm_in[1:NCORES, :],
                    in_=warm_in[0:1, :].to_broadcast((NCORES - 1, 16)),
                )
                nc.gpsimd.collective_compute(
                    "AllToAll",
                    mybir.AluOpType.bypass,
                    replica_groups=[list(range(NCORES))],
                    ins=[warm_in[:].opt()],
                    outs=[warm_out[:].opt()],
                )
                a2a_in = dram_pool.tile([DIM, RPC], bf16, name="a2a_in")
                a2a_out = dram_pool.tile([DIM, RPC], bf16, name="a2a_out")

                def emit_normalize(blk, po_blk):
                    # reciprocal of the denominators via exp(-ln(d)) on ACT
                    # (1 elem/cycle/lane; DVE reciprocal is ~9x slower), PE
                    # broadcast to 64 partitions (psum slots tag-shared with
                    # the S tiles), normalize (bf16), and stage this block
                    # directly into the AllToAll input.
                    q0 = blk * 512
                    lden = nrm_pool.tile([65, 1024], f32, name="lden", tag="lden")
                    nc.scalar.activation(lden[64:65, :], po_blk[64:65, :], LN)
                    sden = nrm_pool.tile([65, 1024], f32r, name="sden", tag="sden")
                    with nc.allow_low_precision(reason="softmax denom"):
                        nc.scalar.activation(
                            sden[64:65, :], lden[64:65, :], EXP, scale=-1.0
                        )
                    for hh in range(HPC):
                        pbc = ps_st.tile([64, 512], f32, name="pbc", tag="st")
                        nc.tensor.matmul(
                            pbc[:],
                            lhsT=ones64[64:65, :],
                            rhs=sden[64 : 65, hh * 512 : (hh + 1) * 512],
                            start=True,
                            stop=True,
                        )
                        bcs = nrm_pool.tile([64, 512], f32, name="bcs", tag="bcs")
                        nc.vector.tensor_copy(out=bcs[:], in_=pbc[:])
                        nc.vector.tensor_mul(
                            out=oT[hh][:, q0 : q0 + 512],
                            in0=po_blk[0:64, hh * 512 : (hh + 1) * 512],
                            in1=bcs[:],
                        )
                        nc.sync.dma_start(
                            out=a2a_in[
                                blk * 128 + hh * 64 : blk * 128 + (hh + 1) * 64, :
                            ],
                            in_=oT[hh][:, q0 : q0 + 512],
                        )

                pending = None  # (blk, po) awaiting normalize
                for b in range(B):
                    for qb in range(NQB):
                        q0 = b * S + qb * 512
                        po = ps_o.tile([65, 1024], f32, name="po", tag="po")
                        for kt in range(NKT):
                            k0 = b * S + kt * 128
                            # both heads' S.T blocks into one 2-bank psum
                            # tile; explicit row-strip tile_position so the
                            # two K=64 matmuls run concurrently on the array
                            pst = ps_st.tile([128, 1024], f32, name="st", tag="st")
                            for hh in range(HPC):
                                nc.tensor.matmul(
                                    pst[:, hh * 512 : (hh + 1) * 512],
                                    lhsT=kT[hh * 64 : (hh + 1) * 64, k0 : k0 + 128],
                                    rhs=qT[hh * 64 : (hh + 1) * 64, q0 : q0 + 512],
                                    start=True,
                                    stop=True,
                                    tile_position=(hh * 64, 0),
                                )
                            ptile = pt_pool.tile(
                                [128, 1024], bf16, name="ptile", tag="ptile"
                            )
                            nc.scalar.activation(ptile[:], pst[:], EXP, scale=SCALE)
                            for hh in range(HPC):
                                col = ((b * HPC + hh) * NKT + kt) * 65
                                nc.tensor.matmul(
                                    po[:, hh * 512 : (hh + 1) * 512],
                                    lhsT=v_nat[:, col : col + 65],
                                    rhs=ptile[:, hh * 512 : (hh + 1) * 512],
                                    start=(kt == 0),
                                    stop=(kt == NKT - 1),
                                )
                            if kt == 6 and pending is not None:
                                emit_normalize(*pending)
                                pending = None
                        pending = (b * NQB + qb, po)
                emit_normalize(*pending)

            # ---------------- stage D: exchange + out projection ----------
            with (
                tc.tile_pool(name="wosb", bufs=1) as wo_pool,
                tc.tile_pool(name="ots", bufs=1) as ots_pool,
                tc.tile_pool(name="psout", bufs=2, space="PSUM") as ps_out,
                tc.tile_pool(name="outt", bufs=2) as out_pool,
            ):
                wo_sb = []
                for k in range(KT):
                    w_t = wo_pool.tile([128, DIM], bf16, name=f"wo_{k}")
                    nc.gpsimd.dma_start(
                        out=w_t[:], in_=wo[k * 128 : (k + 1) * 128, :]
                    )
                    wo_sb.append(w_t)

                nc.gpsimd.collective_compute(
                    "AllToAll",
                    mybir.AluOpType.bypass,
                    replica_groups=[list(range(NCORES))],
                    ins=[a2a_in[:].opt()],
                    outs=[a2a_out[:].opt()],
                )
                oTs = []
                for k in range(KT):
                    o_t = ots_pool.tile([128, RPC], bf16, name=f"oTs_{k}")
                    nc.sync.dma_start(
                        out=o_t[:], in_=a2a_out[k * 128 : (k + 1) * 128, :]
                    )
                    oTs.append(o_t)
                for m in range(8):
                    pout = ps_out.tile([128, 512], f32, name="pout", tag="pout")
                    for k in range(KT):
                        nc.tensor.matmul(
                            pout[:],
                            lhsT=wo_sb[k][:, m * 128 : (m + 1) * 128],
                            rhs=oTs[k][:],
                            start=(k == 0),
                            stop=(k == KT - 1),
                        )
                    o_sb = out_pool.tile([128, 512], f32, name="o_sb", tag="o_sb")
                    nc.vector.tensor_scalar_add(
                        out=o_sb[:], in0=pout[:], scalar1=bias_o[:, m : m + 1]
                    )
                    nc.sync.dma_start(out=out[m * 128 : (m + 1) * 128, :], in_=o_sb[:])

    nc.compile()
    return nc


def _get_graph():
    if "nc" not in _CACHED:
        _CACHED["nc"] = _build_graph()
    return _CACHED["nc"]


def _make_in_maps(x, wqkv, bqkv, wo, bo):
    bf = ml_dtypes.bfloat16
    x2 = np.asarray(x, dtype=np.float32).reshape(R, DIM)
    xt = np.ascontiguousarray(x2.T.astype(bf))  # [dim, b*s] bf16
    wqkv = np.asarray(wqkv, dtype=np.float32)
    bqkv = np.asarray(bqkv, dtype=np.float32)
    wo16 = np.ascontiguousarray(np.asarray(wo, dtype=np.float32).astype(bf))
    bo_f = np.ascontiguousarray(np.asarray(bo, dtype=np.float32).reshape(8, 128))

    in_maps = []
    for c in range(NCORES):
        w_s = np.ascontiguousarray(
            np.concatenate(
                [
                    wqkv[:, c * FPC : (c + 1) * FPC],
                    wqkv[:, DIM + c * FPC : DIM + (c + 1) * FPC],
                    wqkv[:, 2 * DIM + c * FPC : 2 * DIM + (c + 1) * FPC],
                ],
                axis=1,
            ).astype(bf)
        )
        b_s = np.ascontiguousarray(
            np.stack(
                [
                    bqkv[c * FPC : (c + 1) * FPC],
                    bqkv[DIM + c * FPC : DIM + (c + 1) * FPC],
                    bqkv[2 * DIM + c * FPC : 2 * DIM + (c + 1) * FPC],
                ],
                axis=0,
            )
        )
        in_maps.append({"xt": xt, "wqkv": w_s, "bqkv": b_s, "wo": wo16, "bo": bo_f})
    return in_maps


def kernel(x, wqkv, bqkv, wo, bo):
    from concourse.bass_utils import run_bass_kernel_spmd

    nc = _get_graph()
    in_maps = _make_in_maps(x, wqkv, bqkv, wo, bo)
    res = run_bass_kernel_spmd(nc, in_maps, core_ids=list(range(NCORES)))
    outs = [res.results[c]["out"] for c in range(NCORES)]  # each [1024, 512]
    full = np.concatenate([o.T for o in outs], axis=0)  # [4096, 1024]
    return np.ascontiguousarray(full.reshape(B, S, DIM)).astype(np.float32)



# revision 6
# speedup vs baseline: 1.0440x; 1.0440x over previous
"""Distributed multi-head attention kernel for 8 TRN2 NeuronCores (v2).

Sharding: tensor-parallel over heads (2 heads/core). Per core: qkv projection
for its 128 features, attention for its 2 heads, AllToAll exchange, then
row-parallel output projection (each core produces a transposed 512-row slice
of the final output); host reassembles.

v2 structure (vs the v1 baseline):
- V is projected directly in [rows, feat] layout (stationary = x k-tile,
  M = 128 rows) which removes all PE transposes from the V path.
- PV uses column-split tile_position packing: both heads' PV matmuls run
  concurrently in column halves of the PE array (one 512-cycle pass per key
  tile instead of two M=65 passes).
- Softmax denominators come from a bf16 pair/quad/hex DVE reduction tree over
  the exp tiles plus 4 all-ones matmuls per block; reciprocals are computed
  on ACT as exp(-ln(den)) over a [128, 1024] tile whose partition layout
  matches po, so normalization is two plain DVE multiplies (no broadcast
  matmul, no PSUM->SBUF copy).
- The batch-1 qkv projection is interleaved into the PE slack of batch-0's
  attention (the attention phase is ACT-bound), and the emission is software
  pipelined (QK/exp of tile kt ahead of PV of kt-1) so the scalar engine
  never waits on the PE.
- The AllToAll is emitted directly after the last staging DMA.

Compute in bf16 on the PE array (f32 PSUM accumulation, f32 softmax
denominators/normalization). The host pre-transposes x to [dim, b*s] and
pre-casts x/wqkv/wo to bf16 as part of sharding/layout prep.
"""

import sys

sys.path.insert(0, "/opt/trn_rl_repo")

import ml_dtypes
import numpy as np

# Problem constants (hardcoded per harness contract)
B = 2
S = 2048
DIM = 1024
N_HEAD = 16
HD = 64  # head dim
SCALE = HD ** (-0.5)
R = B * S  # 4096 flattened rows
NCORES = 8
HPC = N_HEAD // NCORES  # 2 heads per core
FPC = HPC * HD  # 128 features per core
RPC = R // NCORES  # 512 rows per core (output row slice)

KT = DIM // 128  # 8 k-tiles over the model dim
NKT = S // 128  # 16 key tiles per sequence
NQB = S // 512  # 4 query blocks per sequence

_CACHED = {}


def _build_graph():
    import concourse.mybir as mybir
    import concourse.tile as tile
    from concourse import bacc

    # This kernel only uses Exp and Ln activations. Both live in the
    # "natural_log_exp_and_others" ACT table set, but the table-load pass
    # maps each function to the first set containing it, which puts Exp in
    # "exp_and_others" and forces a ~1.3us table reload around every Ln.
    # Restrict the table list for this build so both resolve to the same
    # set and the table loads exactly once.
    real_get_tables = bacc.get_activation_tables

    def _tables_ln_exp_merged(arch):
        tables = dict(real_get_tables(arch))
        if "natural_log_exp_and_others" in tables and "exp_and_others" in tables:
            tables["exp_and_others"] = set()
        return tables

    nc = bacc.Bacc(
        "TRN2",
        target_bir_lowering=False,
        debug=False,
        num_devices=NCORES,
    )

    bacc.get_activation_tables = _tables_ln_exp_merged
    try:
        _nc = _build_body(nc, mybir, tile)
    finally:
        bacc.get_activation_tables = real_get_tables
    return _nc


def _build_body(nc, mybir, tile):
    f32 = mybir.dt.float32
    bf16 = mybir.dt.bfloat16
    EXP = mybir.ActivationFunctionType.Exp
    LN = mybir.ActivationFunctionType.Ln

    xt = nc.dram_tensor("xt", [DIM, R], bf16, kind="ExternalInput").ap()
    wqkv = nc.dram_tensor("wqkv", [DIM, 3 * FPC], bf16, kind="ExternalInput").ap()
    bqkv = nc.dram_tensor("bqkv", [3, FPC], f32, kind="ExternalInput").ap()
    wo = nc.dram_tensor("wo", [DIM, DIM], bf16, kind="ExternalInput").ap()
    bo = nc.dram_tensor("bo", [8, 128], f32, kind="ExternalInput").ap()
    out = nc.dram_tensor("out", [DIM, RPC], f32, kind="ExternalOutput").ap()

    with tile.TileContext(nc) as tc:
        with (
            tc.tile_pool(name="glob", bufs=1) as glob,
            tc.tile_pool(name="dram", bufs=1, space="DRAM") as dram_pool,
        ):
            # ---------------- persistent tiles -------------------------
            ones128 = glob.tile([128, 128], bf16)
            nc.vector.memset(ones128[:], 1.0)
            bias_qkv = glob.tile([128, 2], f32)  # q, k per-partition biases
            vbias = glob.tile([128, 128], f32)  # v bias along free dim
            bias_o = glob.tile([128, 8], f32)
            qT = glob.tile([128, R], bf16)
            kT = glob.tile([128, R], bf16)
            v_nat = glob.tile([128, R], bf16)  # [keys, 2h*64d] per 128-chunk

            warm_in = dram_pool.tile([NCORES, 16], bf16, name="warm_in")
            warm_out = dram_pool.tile([NCORES, 16], bf16, name="warm_out")
            a2a_in = dram_pool.tile([DIM, RPC], bf16, name="a2a_in")
            a2a_out = dram_pool.tile([DIM, RPC], bf16, name="a2a_out")

            # ---------------- phase 0: weight/bias DMAs, warm a2a ------
            wqkv_sb = []
            for k in range(KT):
                w_t = glob.tile([128, 3 * FPC], bf16, name=f"w_{k}")
                nc.gpsimd.dma_start(out=w_t[:], in_=wqkv[k * 128 : (k + 1) * 128, :])
                wqkv_sb.append(w_t)
            for m in range(2):
                nc.gpsimd.dma_start(
                    out=bias_qkv[:, m : m + 1], in_=bqkv[m : m + 1, :]
                )
            nc.gpsimd.dma_start(
                out=vbias[:], in_=bqkv[2:3, :].to_broadcast((128, 128))
            )
            for m in range(8):
                nc.gpsimd.dma_start(out=bias_o[:, m : m + 1], in_=bo[m : m + 1, :])

            warm_sb = glob.tile([1, 16], bf16)
            nc.vector.memset(warm_sb[:], 1.0)
            nc.gpsimd.dma_start(out=warm_in[0:1, :], in_=warm_sb[0:1, :])
            nc.gpsimd.dma_start(
                out=warm_in[1:NCORES, :],
                in_=warm_in[0:1, :].to_broadcast((NCORES - 1, 16)),
            )
            nc.gpsimd.collective_compute(
                "AllToAll",
                mybir.AluOpType.bypass,
                replica_groups=[list(range(NCORES))],
                ins=[warm_in[:].opt()],
                outs=[warm_out[:].opt()],
            )
            wo_sb = []
            for k in range(KT):
                w_t = glob.tile([128, DIM], bf16, name=f"wo_{k}")
                nc.gpsimd.dma_start(out=w_t[:], in_=wo[k * 128 : (k + 1) * 128, :])
                wo_sb.append(w_t)

            with tc.tile_pool(name="xTp", bufs=2) as xT_pool:

                def dma_group(g):
                    """DMA one 1024-row group of xt; returns the 8 k-tiles."""
                    xg = []
                    for k in range(KT):
                        t = xT_pool.tile(
                            [128, 1024], bf16, name=f"xT_{k}", tag=f"xT{k}"
                        )
                        nc.sync.dma_start(
                            out=t[:],
                            in_=xt[
                                k * 128 : (k + 1) * 128, g * 1024 : (g + 1) * 1024
                            ],
                        )
                        xg.append(t)
                    return xg

                def qk_mms(pp, xg, m, h, ks):
                    for k in ks:
                        nc.tensor.matmul(
                            pp[:],
                            lhsT=wqkv_sb[k][:, m * 128 : (m + 1) * 128],
                            rhs=xg[k][:, h * 512 : (h + 1) * 512],
                            start=(k == 0),
                            stop=(k == KT - 1),
                        )

                def qk_bias(pp, g, m, h):
                    col0 = g * 1024 + h * 512
                    dst = qT if m == 0 else kT
                    nc.vector.tensor_scalar_add(
                        out=dst[:, col0 : col0 + 512],
                        in0=pp[:],
                        scalar1=bias_qkv[:, m : m + 1],
                    )

                def emit_v_chunk(pool, tag, xg, g, c):
                    """Project one 128-row chunk of V directly in natural
                    [rows, feat] layout (stationary = x k-tile columns)."""
                    vd = pool.tile([128, 128], f32, name="vd", tag=tag)
                    for k in range(KT):
                        nc.tensor.matmul(
                            vd[:],
                            lhsT=xg[k][:, c * 128 : (c + 1) * 128],
                            rhs=wqkv_sb[k][:, 256:384],
                            start=(k == 0),
                            stop=(k == KT - 1),
                        )
                    chunk = g * 8 + c
                    nc.vector.tensor_add(
                        out=v_nat[:, chunk * 128 : (chunk + 1) * 128],
                        in0=vd[:],
                        in1=vbias[:],
                    )

                # -------- phase 1: project groups 0, 1 (batch 0) -------
                with (
                    tc.tile_pool(name="pp1", bufs=4, space="PSUM") as pp1_pool,
                    tc.tile_pool(name="vd1", bufs=2, space="PSUM") as vd1_pool,
                ):
                    for g in range(2):
                        xg = dma_group(g)
                        for m in range(2):
                            for h in range(2):
                                pp = pp1_pool.tile(
                                    [128, 512], f32, name="pp", tag="pp"
                                )
                                qk_mms(pp, xg, m, h, range(KT))
                                qk_bias(pp, g, m, h)
                        for c in range(8):
                            emit_v_chunk(vd1_pool, "vd", xg, g, c)

                # -------- phase 2: attention + interleaved g2/g3 proj --
                with (
                    tc.tile_pool(name="pstp", bufs=2, space="PSUM") as pst_pool,
                    tc.tile_pool(name="pop", bufs=1, space="PSUM") as po_pool,
                    tc.tile_pool(name="denp", bufs=1, space="PSUM") as den_pool,
                    tc.tile_pool(name="pp2", bufs=1, space="PSUM") as pp2_pool,
                    tc.tile_pool(name="ptp", bufs=6) as pt_pool,
                    tc.tile_pool(name="pairp", bufs=2) as pair_pool,
                    tc.tile_pool(name="quadp", bufs=2) as quad_pool,
                    tc.tile_pool(name="hexp", bufs=2) as hex_pool,
                    tc.tile_pool(name="ldenp", bufs=1) as lden_pool,
                    tc.tile_pool(name="recipp", bufs=2) as recip_pool,
                    tc.tile_pool(name="oTsp", bufs=2) as oTs_pool,
                ):
                    # Deferred emission units for the g2/g3 projection,
                    # popped one per key-tile through batch-0's attention.
                    st = {"xg": None, "pp": None, "pending": None}
                    units = []
                    for g in (2, 3):

                        def u_dma(g=g):
                            st["xg"] = dma_group(g)

                        units.append(u_dma)
                        for m in range(2):
                            for h in range(2):

                                def u_qk_a(m=m, h=h):
                                    st["pp"] = pp2_pool.tile(
                                        [128, 512], f32, name="pp", tag="pp"
                                    )
                                    qk_mms(st["pp"], st["xg"], m, h, range(4))

                                def u_qk_b(g=g, m=m, h=h):
                                    qk_mms(st["pp"], st["xg"], m, h, range(4, 8))
                                    qk_bias(st["pp"], g, m, h)

                                units.append(u_qk_a)
                                units.append(u_qk_b)
                        for c in range(8):

                            def u_v(g=g, c=c):
                                emit_v_chunk(pp2_pool, "pp", st["xg"], g, c)

                            units.append(u_v)
                    units.reverse()  # so units.pop() emits in order

                    def emit_pv(blk, kt, pts, po, tree):
                        b = blk // NQB
                        off = (b * NKT + kt) * 128
                        pt = pts[kt]
                        nc.tensor.matmul(
                            po[0:64, :],
                            lhsT=v_nat[:, off : off + 64],
                            rhs=pt[:, 0:512],
                            start=(kt == 0),
                            stop=(kt == NKT - 1),
                            tile_position=(0, 0),
                        )
                        nc.tensor.matmul(
                            po[64:128, :],
                            lhsT=v_nat[:, off + 64 : off + 128],
                            rhs=pt[:, 512:1024],
                            start=(kt == 0),
                            stop=(kt == NKT - 1),
                            tile_position=(0, 64),
                        )
                        # bf16 reduction tree toward the denominators
                        if kt % 2 == 1:
                            pr = pair_pool.tile(
                                [128, 1024], bf16, name="pair", tag="pair"
                            )
                            nc.vector.tensor_add(
                                out=pr[:], in0=pts[kt - 1][:], in1=pt[:]
                            )
                            tree["pair"].append(pr)
                        if kt % 4 == 3:
                            qd = quad_pool.tile(
                                [128, 1024], bf16, name="quad", tag="quad"
                            )
                            nc.vector.tensor_add(
                                out=qd[:],
                                in0=tree["pair"][-2][:],
                                in1=tree["pair"][-1][:],
                            )
                            tree["quad"].append(qd)
                        if kt % 8 == 7:
                            hx = hex_pool.tile(
                                [128, 1024], bf16, name="hex", tag="hex"
                            )
                            nc.vector.tensor_add(
                                out=hx[:],
                                in0=tree["quad"][-2][:],
                                in1=tree["quad"][-1][:],
                            )
                            tree["hex"].append(hx)

                    def emit_block_tail(blk, pts, po, tree):
                        emit_pv(blk, NKT - 1, pts, po, tree)
                        hx0, hx1 = tree["hex"]
                        dn = den_pool.tile([128, 1024], f32, name="den", tag="den")
                        for half in range(2):
                            c0 = half * 512
                            nc.tensor.matmul(
                                dn[:, c0 : c0 + 512],
                                lhsT=ones128[:],
                                rhs=hx0[:, c0 : c0 + 512],
                                start=True,
                                stop=False,
                            )
                            nc.tensor.matmul(
                                dn[:, c0 : c0 + 512],
                                lhsT=ones128[:],
                                rhs=hx1[:, c0 : c0 + 512],
                                start=False,
                                stop=True,
                            )
                        lden = lden_pool.tile(
                            [128, 1024], f32, name="lden", tag="ld"
                        )
                        nc.scalar.activation(lden[:], dn[:], LN)
                        recip = recip_pool.tile(
                            [128, 1024], f32, name="recip", tag="rc"
                        )
                        with nc.allow_low_precision(reason="softmax denom"):
                            nc.scalar.activation(recip[:], lden[:], EXP, scale=-1.0)
                        oTs = oTs_pool.tile([128, 512], bf16, name="oTs", tag="oTs")
                        nc.vector.tensor_mul(
                            out=oTs[0:64, :],
                            in0=po[0:64, :],
                            in1=recip[0:64, 0:512],
                        )
                        nc.vector.tensor_mul(
                            out=oTs[64:128, :],
                            in0=po[64:128, :],
                            in1=recip[64:128, 512:1024],
                        )
                        nc.sync.dma_start(
                            out=a2a_in[blk * 128 : (blk + 1) * 128, :], in_=oTs[:]
                        )

                    for b in range(B):
                        for qb in range(NQB):
                            blk = b * NQB + qb
                            q0 = b * S + qb * 512
                            pts = []
                            tree = {"pair": [], "quad": [], "hex": []}
                            po = None
                            for kt in range(NKT):
                                k0 = b * S + kt * 128
                                pst = pst_pool.tile(
                                    [128, 1024], f32, name="pst", tag="st"
                                )
                                for hh in range(HPC):
                                    nc.tensor.matmul(
                                        pst[:, hh * 512 : (hh + 1) * 512],
                                        lhsT=kT[
                                            hh * 64 : (hh + 1) * 64, k0 : k0 + 128
                                        ],
                                        rhs=qT[
                                            hh * 64 : (hh + 1) * 64, q0 : q0 + 512
                                        ],
                                        start=True,
                                        stop=True,
                                        tile_position=(hh * 64, 0),
                                    )
                                pt = pt_pool.tile(
                                    [128, 1024], bf16, name="ptile", tag="pt"
                                )
                                nc.scalar.activation(pt[:], pst[:], EXP, scale=SCALE)
                                pts.append(pt)
                                if kt == 0:
                                    # finish the previous block behind this
                                    # block's first QK/exp so ACT stays fed
                                    if st["pending"] is not None:
                                        emit_block_tail(*st["pending"])
                                    po = po_pool.tile(
                                        [128, 512], f32, name="po", tag="po"
                                    )
                                else:
                                    emit_pv(blk, kt - 1, pts, po, tree)
                                if blk < 4 and units:
                                    units.pop()()
                            st["pending"] = (blk, pts, po, tree)
                    # flush the last block and launch the exchange
                    emit_block_tail(*st["pending"])
                    while units:
                        units.pop()()
                    nc.gpsimd.collective_compute(
                        "AllToAll",
                        mybir.AluOpType.bypass,
                        replica_groups=[list(range(NCORES))],
                        ins=[a2a_in[:].opt()],
                        outs=[a2a_out[:].opt()],
                    )

            # ---------------- phase 3: output projection ---------------
            with (
                tc.tile_pool(name="ots", bufs=1) as ots_pool,
                tc.tile_pool(name="psout", bufs=2, space="PSUM") as ps_out,
                tc.tile_pool(name="outt", bufs=2) as out_pool,
            ):
                oTs_full = []
                for k in range(KT):
                    o_t = ots_pool.tile([128, RPC], bf16, name=f"oTs_{k}")
                    nc.sync.dma_start(
                        out=o_t[:], in_=a2a_out[k * 128 : (k + 1) * 128, :]
                    )
                    oTs_full.append(o_t)
                for m in range(8):
                    pout = ps_out.tile([128, 512], f32, name="pout", tag="pout")
                    for k in range(KT):
                        nc.tensor.matmul(
                            pout[:],
                            lhsT=wo_sb[k][:, m * 128 : (m + 1) * 128],
                            rhs=oTs_full[k][:],
                            start=(k == 0),
                            stop=(k == KT - 1),
                        )
                    o_sb = out_pool.tile([128, 512], f32, name="o_sb", tag="o_sb")
                    nc.vector.tensor_scalar_add(
                        out=o_sb[:], in0=pout[:], scalar1=bias_o[:, m : m + 1]
                    )
                    nc.sync.dma_start(
                        out=out[m * 128 : (m + 1) * 128, :], in_=o_sb[:]
                    )

    nc.compile()
    return nc


def _get_graph():
    if "nc" not in _CACHED:
        _CACHED["nc"] = _build_graph()
    return _CACHED["nc"]


def _make_in_maps(x, wqkv, bqkv, wo, bo):
    bf = ml_dtypes.bfloat16
    x2 = np.asarray(x, dtype=np.float32).reshape(R, DIM)
    xt = np.ascontiguousarray(x2.T.astype(bf))  # [dim, b*s] bf16
    wqkv = np.asarray(wqkv, dtype=np.float32)
    bqkv = np.asarray(bqkv, dtype=np.float32)
    wo16 = np.ascontiguousarray(np.asarray(wo, dtype=np.float32).astype(bf))
    bo_f = np.ascontiguousarray(np.asarray(bo, dtype=np.float32).reshape(8, 128))

    in_maps = []
    for c in range(NCORES):
        w_s = np.ascontiguousarray(
            np.concatenate(
                [
                    wqkv[:, c * FPC : (c + 1) * FPC],
                    wqkv[:, DIM + c * FPC : DIM + (c + 1) * FPC],
                    wqkv[:, 2 * DIM + c * FPC : 2 * DIM + (c + 1) * FPC],
                ],
                axis=1,
            ).astype(bf)
        )
        b_s = np.ascontiguousarray(
            np.stack(
                [
                    bqkv[c * FPC : (c + 1) * FPC],
                    bqkv[DIM + c * FPC : DIM + (c + 1) * FPC],
                    bqkv[2 * DIM + c * FPC : 2 * DIM + (c + 1) * FPC],
                ],
                axis=0,
            )
        )
        in_maps.append({"xt": xt, "wqkv": w_s, "bqkv": b_s, "wo": wo16, "bo": bo_f})
    return in_maps


def kernel(x, wqkv, bqkv, wo, bo):
    from concourse.bass_utils import run_bass_kernel_spmd

    nc = _get_graph()
    in_maps = _make_in_maps(x, wqkv, bqkv, wo, bo)
    res = run_bass_kernel_spmd(nc, in_maps, core_ids=list(range(NCORES)))
    outs = [res.results[c]["out"] for c in range(NCORES)]  # each [1024, 512]
    full = np.concatenate([o.T for o in outs], axis=0)  # [4096, 1024]
    return np.ascontiguousarray(full.reshape(B, S, DIM)).astype(np.float32)


# revision 11
# speedup vs baseline: 1.1316x; 1.0839x over previous
"""Distributed multi-head attention kernel for 8 TRN2 NeuronCores (v3).

Sharding: tensor-parallel over heads (2 heads/core). Per core: qkv projection
for its 128 features, attention for its 2 heads, AllToAll exchange, then
row-parallel output projection (each core produces a transposed 512-row slice
of the final output); host reassembles.

Structure:
- V is projected directly in [rows, feat] layout (stationary = x k-tile,
  M = 128 rows): no PE transposes anywhere.
- PV uses column-split tile_position packing: both heads' PV matmuls run
  concurrently in column halves of the PE array (one 512-cycle pass per key
  tile instead of two M=65 passes).
- Softmax denominators come from a bf16 pair/quad/hex DVE reduction tree over
  the exp tiles plus 4 all-ones matmuls per block; reciprocals via the DVE
  reciprocal_approx_fast custom op (the scalar engine runs nothing but the
  128 exp tiles); normalization is two plain DVE multiplies whose recip
  operand layout matches po's partition split.
- The attention phase is ACT(exp)-bound, so everything else is threaded
  through its PE slack: the emission is software-pipelined (QK/exp of tile
  kt runs SHIFT tiles ahead of PV), the previous block's denominator/
  normalize work is spread over the next block's first tiles, and the whole
  remaining projection work (q for later blocks, all of batch 1) is emitted
  as fine-grained units popped between key tiles with deadline ordering.
- A dependency-chained dummy-matmul ladder keeps the PE activity monitor
  warm during the AllToAll so the output projection runs at full clock; the
  output projection is k-outer so it starts as soon as the first received
  k-tile lands.

Compute in bf16 on the PE array (f32 PSUM accumulation, f32 softmax
denominators/normalization). The host pre-transposes x to [dim, b*s] and
pre-casts x/wqkv/wo to bf16 as part of sharding/layout prep.
"""

import sys

sys.path.insert(0, "/opt/trn_rl_repo")

import ml_dtypes
import numpy as np

# Problem constants (hardcoded per harness contract)
B = 2
S = 2048
DIM = 1024
N_HEAD = 16
HD = 64  # head dim
SCALE = HD ** (-0.5)
R = B * S  # 4096 flattened rows
NCORES = 8
HPC = N_HEAD // NCORES  # 2 heads per core
FPC = HPC * HD  # 128 features per core
RPC = R // NCORES  # 512 rows per core (output row slice)

KT = DIM // 128  # 8 k-tiles over the model dim
NKT = S // 128  # 16 key tiles per sequence
NQB = S // 512  # 4 query blocks per sequence
SHIFT = 3  # PV pipeline lag behind QK/exp

_CACHED = {}


def _build_graph():
    import concourse.mybir as mybir
    import concourse.tile as tile
    from concourse import bacc

    nc = bacc.Bacc(
        "TRN2",
        target_bir_lowering=False,
        debug=False,
        num_devices=NCORES,
    )
    return _build_body(nc, mybir, tile)


def _build_body(nc, mybir, tile):
    f32 = mybir.dt.float32
    bf16 = mybir.dt.bfloat16
    EXP = mybir.ActivationFunctionType.Exp

    xt = nc.dram_tensor("xt", [DIM, R], bf16, kind="ExternalInput").ap()
    wqkv = nc.dram_tensor("wqkv", [DIM, 3 * FPC], bf16, kind="ExternalInput").ap()
    bqkv = nc.dram_tensor("bqkv", [3, FPC], f32, kind="ExternalInput").ap()
    wo = nc.dram_tensor("wo", [DIM, DIM], bf16, kind="ExternalInput").ap()
    bo = nc.dram_tensor("bo", [8, 128], f32, kind="ExternalInput").ap()
    out = nc.dram_tensor("out", [DIM, RPC], f32, kind="ExternalOutput").ap()

    with tile.TileContext(nc) as tc:
        with (
            tc.tile_pool(name="glob", bufs=1) as glob,
            tc.tile_pool(name="dram", bufs=1, space="DRAM") as dram_pool,
        ):
            # ---------------- persistent tiles -------------------------
            ones128 = glob.tile([128, 128], bf16)
            nc.vector.memset(ones128[:], 1.0)
            bias_qkv = glob.tile([128, 2], f32)  # q, k per-partition biases
            vbias = glob.tile([128, 128], f32)  # v bias along free dim
            bias_o = glob.tile([128, 8], f32)
            qT = glob.tile([128, R], bf16)
            kT = glob.tile([128, R], bf16)
            v_nat = glob.tile([128, R], bf16)  # [keys, 2h*64d] per 128-chunk

            warm_in = dram_pool.tile([NCORES, 16], bf16, name="warm_in")
            warm_out = dram_pool.tile([NCORES, 16], bf16, name="warm_out")
            a2a_in = dram_pool.tile([DIM, RPC], bf16, name="a2a_in")
            a2a_out = dram_pool.tile([DIM, RPC], bf16, name="a2a_out")

            # ---------------- phase 0: weight/bias DMAs, warm a2a ------
            wqkv_sb = []
            for k in range(KT):
                w_t = glob.tile([128, 3 * FPC], bf16, name=f"w_{k}")
                nc.gpsimd.dma_start(out=w_t[:], in_=wqkv[k * 128 : (k + 1) * 128, :])
                wqkv_sb.append(w_t)
            for m in range(2):
                nc.gpsimd.dma_start(
                    out=bias_qkv[:, m : m + 1], in_=bqkv[m : m + 1, :]
                )
            nc.gpsimd.dma_start(
                out=vbias[:], in_=bqkv[2:3, :].to_broadcast((128, 128))
            )
            for m in range(8):
                nc.gpsimd.dma_start(out=bias_o[:, m : m + 1], in_=bo[m : m + 1, :])

            warm_sb = glob.tile([1, 16], bf16)
            nc.vector.memset(warm_sb[:], 1.0)
            nc.gpsimd.dma_start(out=warm_in[0:1, :], in_=warm_sb[0:1, :])
            nc.gpsimd.dma_start(
                out=warm_in[1:NCORES, :],
                in_=warm_in[0:1, :].to_broadcast((NCORES - 1, 16)),
            )
            nc.gpsimd.collective_compute(
                "AllToAll",
                mybir.AluOpType.bypass,
                replica_groups=[list(range(NCORES))],
                ins=[warm_in[:].opt()],
                outs=[warm_out[:].opt()],
            )
            wo_sb = []
            for k in range(KT):
                w_t = glob.tile([128, DIM], bf16, name=f"wo_{k}")
                nc.gpsimd.dma_start(out=w_t[:], in_=wo[k * 128 : (k + 1) * 128, :])
                wo_sb.append(w_t)

            with tc.tile_pool(name="xTp", bufs=2) as xT_pool:

                def dma_group(g):
                    """DMA one 1024-row group of xt; returns the 8 k-tiles."""
                    xg = []
                    for k in range(KT):
                        t = xT_pool.tile(
                            [128, 1024], bf16, name=f"xT_{k}", tag=f"xT{k}"
                        )
                        nc.sync.dma_start(
                            out=t[:],
                            in_=xt[
                                k * 128 : (k + 1) * 128, g * 1024 : (g + 1) * 1024
                            ],
                        )
                        xg.append(t)
                    return xg

                def qk_mms(pp, xg, m, h, ks):
                    for k in ks:
                        nc.tensor.matmul(
                            pp[:],
                            lhsT=wqkv_sb[k][:, m * 128 : (m + 1) * 128],
                            rhs=xg[k][:, h * 512 : (h + 1) * 512],
                            start=(k == 0),
                            stop=(k == KT - 1),
                        )

                def qk_bias(pp, g, m, h):
                    col0 = g * 1024 + h * 512
                    dst = qT if m == 0 else kT
                    nc.vector.tensor_scalar_add(
                        out=dst[:, col0 : col0 + 512],
                        in0=pp[:],
                        scalar1=bias_qkv[:, m : m + 1],
                    )

                def v_mms(vd, xg, c, ks):
                    for k in ks:
                        nc.tensor.matmul(
                            vd[:],
                            lhsT=xg[k][:, c * 128 : (c + 1) * 128],
                            rhs=wqkv_sb[k][:, 256:384],
                            start=(k == 0),
                            stop=(k == KT - 1),
                        )

                def v_bias(vd, g, c):
                    chunk = g * 8 + c
                    nc.vector.tensor_add(
                        out=v_nat[:, chunk * 128 : (chunk + 1) * 128],
                        in0=vd[:],
                        in1=vbias[:],
                    )

                # -------- phase 1 prefix: just enough to start block 0 --
                xgs = {}
                with (
                    tc.tile_pool(name="pp1", bufs=2, space="PSUM") as pp1_pool,
                    tc.tile_pool(name="vd1", bufs=1, space="PSUM") as vd1_pool,
                ):
                    xgs[0] = dma_group(0)
                    xgs[1] = dma_group(1)
                    pp = pp1_pool.tile([128, 512], f32, name="pp", tag="pp")
                    qk_mms(pp, xgs[0], 1, 0, range(KT))  # k, rows 0-511
                    qk_bias(pp, 0, 1, 0)
                    vd = vd1_pool.tile([128, 128], f32, name="vd", tag="vd")
                    v_mms(vd, xgs[0], 0, range(KT))  # v chunk 0
                    v_bias(vd, 0, 0)
                    pp = pp1_pool.tile([128, 512], f32, name="pp", tag="pp")
                    qk_mms(pp, xgs[0], 0, 0, range(KT))  # q, rows 0-511
                    qk_bias(pp, 0, 0, 0)

                # -------- phase 2: attention + interleaved projection --
                with (
                    tc.tile_pool(name="pstp", bufs=2, space="PSUM") as pst_pool,
                    tc.tile_pool(name="pop", bufs=1, space="PSUM") as po_pool,
                    tc.tile_pool(name="denp", bufs=1, space="PSUM") as den_pool,
                    tc.tile_pool(name="pp2", bufs=1, space="PSUM") as pp2_pool,
                    tc.tile_pool(name="ptp", bufs=6) as pt_pool,
                    tc.tile_pool(name="pairp", bufs=2) as pair_pool,
                    tc.tile_pool(name="quadp", bufs=2) as quad_pool,
                    tc.tile_pool(name="hexp", bufs=2) as hex_pool,
                    tc.tile_pool(name="recipp", bufs=2) as recip_pool,
                    tc.tile_pool(name="oTsp", bufs=2) as oTs_pool,
                    tc.tile_pool(name="jsbp", bufs=1) as jsb_pool,
                ):
                    st = {"pp": None, "vd": None, "pending": None}

                    # ---- deferred projection units (deadline-ordered) --
                    def u_dma(g):
                        return lambda: xgs.__setitem__(g, dma_group(g))

                    def u_round_start(g, m, h, pool):
                        def f():
                            st["pp"] = pool.tile(
                                [128, 512], f32, name="pp", tag="pp"
                            )
                            qk_mms(st["pp"], xgs[g], m, h, range(2))

                        return f

                    def u_round_mid(g, m, h, ks):
                        return lambda: qk_mms(st["pp"], xgs[g], m, h, ks)

                    def u_round_end(g, m, h):
                        def f():
                            qk_mms(st["pp"], xgs[g], m, h, range(6, 8))
                            qk_bias(st["pp"], g, m, h)

                        return f

                    def round_units(g, m, h, pool):
                        return [
                            u_round_start(g, m, h, pool),
                            u_round_mid(g, m, h, range(2, 4)),
                            u_round_mid(g, m, h, range(4, 6)),
                            u_round_end(g, m, h),
                        ]

                    def u_v_a(g, c, pool):
                        def f():
                            st["vd"] = pool.tile(
                                [128, 128], f32, name="vd", tag="pp"
                            )
                            v_mms(st["vd"], xgs[g], c, range(4))

                        return f

                    def u_v_b(g, c):
                        def f():
                            v_mms(st["vd"], xgs[g], c, range(4, 8))
                            v_bias(st["vd"], g, c)

                        return f

                    def v_units(g, c, pool):
                        return [u_v_a(g, c, pool), u_v_b(g, c)]

                    p2 = pp2_pool
                    units = []
                    # batch-0 remainder, deadline-interleaved for block 0
                    # (3 pops/kt): v chunk c is needed by PV at kt c+SHIFT;
                    # k-round (g,h) covers key tiles g*8+h*4 .. +3.
                    units += v_units(0, 1, p2) + v_units(0, 2, p2)
                    units += round_units(0, 1, 1, p2)  # k rows 512-1023
                    units += v_units(0, 3, p2) + v_units(0, 4, p2)
                    units += round_units(1, 1, 0, p2)  # k rows 1024-1535
                    units += v_units(0, 5, p2) + v_units(0, 6, p2)
                    units += round_units(1, 1, 1, p2)  # k rows 1536-2047
                    units += v_units(0, 7, p2)
                    for c in range(8):
                        units += v_units(1, c, p2)  # v rows 1024-2047
                    units += round_units(0, 0, 1, p2)  # q for block 1
                    units += round_units(1, 0, 0, p2)  # q for block 2
                    units += round_units(1, 0, 1, p2)  # q for block 3
                    # batch 1: k and v (needed by block 4), q(g2,h0) too
                    units += [u_dma(2)]
                    units += round_units(2, 1, 0, p2) + round_units(2, 1, 1, p2)
                    for c in range(4):
                        units += v_units(2, c, p2)
                    units += [u_dma(3)]
                    for c in range(4, 8):
                        units += v_units(2, c, p2)
                    units += round_units(3, 1, 0, p2) + round_units(3, 1, 1, p2)
                    for c in range(8):
                        units += v_units(3, c, p2)
                    units += round_units(2, 0, 0, p2)  # q for block 4
                    # popped during blocks 4-6:
                    late_units = (
                        round_units(2, 0, 1, p2)  # q block 5
                        + round_units(3, 0, 0, p2)  # q block 6
                        + round_units(3, 0, 1, p2)  # q block 7
                    )
                    units.reverse()
                    late_units.reverse()

                    def emit_pv(blk, kt, pts, po, tree):
                        b = blk // NQB
                        off = (b * NKT + kt) * 128
                        pt = pts[kt]
                        nc.tensor.matmul(
                            po[0:64, :],
                            lhsT=v_nat[:, off : off + 64],
                            rhs=pt[:, 0:512],
                            start=(kt == 0),
                            stop=(kt == NKT - 1),
                            tile_position=(0, 0),
                        )
                        nc.tensor.matmul(
                            po[64:128, :],
                            lhsT=v_nat[:, off + 64 : off + 128],
                            rhs=pt[:, 512:1024],
                            start=(kt == 0),
                            stop=(kt == NKT - 1),
                            tile_position=(0, 64),
                        )
                        # bf16 reduction tree toward the denominators
                        if kt % 2 == 1:
                            pr = pair_pool.tile(
                                [128, 1024], bf16, name="pair", tag="pair"
                            )
                            nc.vector.tensor_add(
                                out=pr[:], in0=pts[kt - 1][:], in1=pt[:]
                            )
                            tree["pair"].append(pr)
                        if kt % 4 == 3:
                            qd = quad_pool.tile(
                                [128, 1024], bf16, name="quad", tag="quad"
                            )
                            nc.vector.tensor_add(
                                out=qd[:],
                                in0=tree["pair"][-2][:],
                                in1=tree["pair"][-1][:],
                            )
                            tree["quad"].append(qd)
                        if kt % 8 == 7:
                            hx = hex_pool.tile(
                                [128, 1024], bf16, name="hex", tag="hex"
                            )
                            nc.vector.tensor_add(
                                out=hx[:],
                                in0=tree["quad"][-2][:],
                                in1=tree["quad"][-1][:],
                            )
                            tree["hex"].append(hx)

                    def tail_a(blk, pts, po, tree):
                        for kt in range(NKT - SHIFT, NKT):
                            emit_pv(blk, kt, pts, po, tree)

                    def tail_b(blk, pts, po, tree):
                        # denominator part 1: hex0 into both halves (start)
                        dn = den_pool.tile([128, 1024], f32, name="den", tag="den")
                        st["den"] = dn
                        hx0 = tree["hex"][0]
                        for half in range(2):
                            c0 = half * 512
                            nc.tensor.matmul(
                                dn[:, c0 : c0 + 512],
                                lhsT=ones128[:],
                                rhs=hx0[:, c0 : c0 + 512],
                                start=True,
                                stop=False,
                            )

                    def tail_c(blk, pts, po, tree):
                        dn = st["den"]
                        hx1 = tree["hex"][1]
                        for half in range(2):
                            c0 = half * 512
                            nc.tensor.matmul(
                                dn[:, c0 : c0 + 512],
                                lhsT=ones128[:],
                                rhs=hx1[:, c0 : c0 + 512],
                                start=False,
                                stop=True,
                            )
                        recip = recip_pool.tile(
                            [128, 1024], f32, name="recip", tag="rc"
                        )
                        nc.vector.reciprocal_approx_fast(out=recip[:], in_=dn[:])
                        oTs = oTs_pool.tile([128, 512], bf16, name="oTs", tag="oTs")
                        nc.vector.tensor_mul(
                            out=oTs[0:64, :],
                            in0=po[0:64, :],
                            in1=recip[0:64, 0:512],
                        )
                        nc.vector.tensor_mul(
                            out=oTs[64:128, :],
                            in0=po[64:128, :],
                            in1=recip[64:128, 512:1024],
                        )
                        nc.sync.dma_start(
                            out=a2a_in[blk * 128 : (blk + 1) * 128, :], in_=oTs[:]
                        )

                    for b in range(B):
                        for qb in range(NQB):
                            blk = b * NQB + qb
                            q0 = b * S + qb * 512
                            pts = []
                            tree = {"pair": [], "quad": [], "hex": []}
                            po = None
                            for kt in range(NKT):
                                k0 = b * S + kt * 128
                                pst = pst_pool.tile(
                                    [128, 1024], f32, name="pst", tag="st"
                                )
                                for hh in range(HPC):
                                    nc.tensor.matmul(
                                        pst[:, hh * 512 : (hh + 1) * 512],
                                        lhsT=kT[
                                            hh * 64 : (hh + 1) * 64, k0 : k0 + 128
                                        ],
                                        rhs=qT[
                                            hh * 64 : (hh + 1) * 64, q0 : q0 + 512
                                        ],
                                        start=True,
                                        stop=True,
                                        tile_position=(hh * 64, 0),
                                    )
                                pt = pt_pool.tile(
                                    [128, 1024], bf16, name="ptile", tag="pt"
                                )
                                nc.scalar.activation(pt[:], pst[:], EXP, scale=SCALE)
                                pts.append(pt)
                                pend = st["pending"]
                                if kt == 0 and pend:
                                    tail_a(*pend)
                                elif kt == 1 and pend:
                                    tail_b(*pend)
                                elif kt == 2 and pend:
                                    tail_c(*pend)
                                    st["pending"] = None
                                if kt == SHIFT:
                                    po = po_pool.tile(
                                        [128, 512], f32, name="po", tag="po"
                                    )
                                if kt >= SHIFT:
                                    emit_pv(blk, kt - SHIFT, pts, po, tree)
                                # deadline-paced unit pops
                                npop = 0
                                if blk == 0:
                                    npop = 3
                                elif blk < 4:
                                    npop = 2 if kt % 4 == 0 else 1
                                elif blk < 7:
                                    npop = 1 if kt % 4 == 0 else 0
                                for _ in range(npop):
                                    if blk < 4 and units:
                                        units.pop()()
                                    elif late_units:
                                        late_units.pop()()
                            st["pending"] = (blk, pts, po, tree)
                    # flush the last block, then keep-warm + exchange
                    tail_a(*st["pending"])
                    tail_b(*st["pending"])
                    tail_c(*st["pending"])
                    st["pending"] = None
                    while units:
                        units.pop()()
                    while late_units:
                        late_units.pop()()
                    nc.gpsimd.collective_compute(
                        "AllToAll",
                        mybir.AluOpType.bypass,
                        replica_groups=[list(range(NCORES))],
                        ins=[a2a_in[:].opt()],
                        outs=[a2a_out[:].opt()],
                    )
                    # dependency-chained dummy matmuls: keep the PE activity
                    # monitor warm across the AllToAll so the output
                    # projection starts at full clock
                    jsb = jsb_pool.tile([128, 512], bf16, name="jsb", tag="jsb")
                    nc.vector.tensor_copy(out=jsb[:], in_=qT[:, 0:512])
                    for _ in range(12):
                        jp = pp2_pool.tile([128, 512], f32, name="jp", tag="pp")
                        nc.tensor.matmul(
                            jp[:], lhsT=ones128[:], rhs=jsb[:],
                            start=True, stop=True,
                        )
                        jsb = jsb_pool.tile([128, 512], bf16, name="jsb", tag="jsb")
                        nc.vector.tensor_copy(out=jsb[:], in_=jp[:])

            # ---------------- phase 3: output projection ---------------
            with (
                tc.tile_pool(name="ots", bufs=1) as ots_pool,
                tc.tile_pool(name="psout", bufs=1, space="PSUM") as ps_out,
                tc.tile_pool(name="outt", bufs=2) as out_pool,
            ):
                pouts = [
                    ps_out.tile([128, 512], f32, name=f"pout{m}", tag=f"po{m}")
                    for m in range(8)
                ]
                for k in range(KT):
                    o_t = ots_pool.tile([128, RPC], bf16, name=f"oTs_{k}")
                    nc.sync.dma_start(
                        out=o_t[:], in_=a2a_out[k * 128 : (k + 1) * 128, :]
                    )
                    for m in range(8):
                        nc.tensor.matmul(
                            pouts[m][:],
                            lhsT=wo_sb[k][:, m * 128 : (m + 1) * 128],
                            rhs=o_t[:],
                            start=(k == 0),
                            stop=(k == KT - 1),
                        )
                for m in range(8):
                    o_sb = out_pool.tile([128, 512], f32, name="o_sb", tag="o_sb")
                    nc.vector.tensor_scalar_add(
                        out=o_sb[:], in0=pouts[m][:], scalar1=bias_o[:, m : m + 1]
                    )
                    nc.sync.dma_start(
                        out=out[m * 128 : (m + 1) * 128, :], in_=o_sb[:]
                    )

    nc.compile()
    return nc


def _get_graph():
    if "nc" not in _CACHED:
        _CACHED["nc"] = _build_graph()
    return _CACHED["nc"]


def _make_in_maps(x, wqkv, bqkv, wo, bo):
    bf = ml_dtypes.bfloat16
    x2 = np.asarray(x, dtype=np.float32).reshape(R, DIM)
    xt = np.ascontiguousarray(x2.T.astype(bf))  # [dim, b*s] bf16
    wqkv = np.asarray(wqkv, dtype=np.float32)
    bqkv = np.asarray(bqkv, dtype=np.float32)
    wo16 = np.ascontiguousarray(np.asarray(wo, dtype=np.float32).astype(bf))
    bo_f = np.ascontiguousarray(np.asarray(bo, dtype=np.float32).reshape(8, 128))

    in_maps = []
    for c in range(NCORES):
        w_s = np.ascontiguousarray(
            np.concatenate(
                [
                    wqkv[:, c * FPC : (c + 1) * FPC],
                    wqkv[:, DIM + c * FPC : DIM + (c + 1) * FPC],
                    wqkv[:, 2 * DIM + c * FPC : 2 * DIM + (c + 1) * FPC],
                ],
                axis=1,
            ).astype(bf)
        )
        b_s = np.ascontiguousarray(
            np.stack(
                [
                    bqkv[c * FPC : (c + 1) * FPC],
                    bqkv[DIM + c * FPC : DIM + (c + 1) * FPC],
                    bqkv[2 * DIM + c * FPC : 2 * DIM + (c + 1) * FPC],
                ],
                axis=0,
            )
        )
        in_maps.append({"xt": xt, "wqkv": w_s, "bqkv": b_s, "wo": wo16, "bo": bo_f})
    return in_maps


def kernel(x, wqkv, bqkv, wo, bo):
    from concourse.bass_utils import run_bass_kernel_spmd

    nc = _get_graph()
    in_maps = _make_in_maps(x, wqkv, bqkv, wo, bo)
    res = run_bass_kernel_spmd(nc, in_maps, core_ids=list(range(NCORES)))
    outs = [res.results[c]["out"] for c in range(NCORES)]  # each [1024, 512]
    full = np.concatenate([o.T for o in outs], axis=0)  # [4096, 1024]
    return np.ascontiguousarray(full.reshape(B, S, DIM)).astype(np.float32)
